# revision 69
# baseline (speedup 1.0000x reference)
"""AttentionBlock (GroupNorm + 1x1-conv QKV self-attention + residual) on 8 TRN2 cores.

Sharding: data-parallel over batch B=4 x sequence-parallel over the 4096
tokens (2 cores per batch element, each handling 2048 query rows; K/V and
GroupNorm are computed redundantly per core pair — they are cheap relative
to attention).

Per-core device kernel (fp8 attention matmuls, GN stats + residual fp32/bf16):
  - x ships as bf16 only (no fp32 copy): matmul/stats operand AND the
    residual base (bf16 residual error ~1e-3 of output absmax, well under
    tolerance). GroupNorm stats overlap the input DMA (bn_stats per piece,
    group-combine via tiny indicator matmuls on the PE).
  - GroupNorm folds into the QKV weights: h = scale_c*x + shift_c, so q/k/v
    come straight from x with per-channel-scaled weights + effective biases.
  - q and k are computed together, 2x-replicated, by a single packed
    stationary [wq|wk|wq|wk] [128,128] matmul per 512-token chunk (one
    moving-data pass instead of four), evacuated once with the interleaved
    bias vector, then partition-rearranged into q_rep/k_rep [64, *] via
    SBUF->SBUF DMAs (free on the DMA engines).
  - S^T: 2x row-packed bf16 matmuls (K=32 contraction) into a 2-bank PSUM
    tile; one [128,1024] exp on the scalar engine writes fp8e4 directly
    (softmax scale fused; S range ~±0.8 so no max subtraction).
  - P*V runs in fp8 DoubleRow mode: each matmul contracts TWO 128-key
    m-blocks per pass (2x PE throughput); V^T is evacuated to fp8.
  - Softmax denominator: one fp8 DoubleRow ones-matmul [128,2,128] per
    group accumulates the column sums replicated across ALL 128 PSUM
    partitions; the epilogue takes a [1,512] reciprocal (tiny) and
    re-broadcasts it into the same PSUM bank with a contraction-1 matmul.
    No mask, no GpSimd partition reduce, no [128,512] reciprocal.
  - The per-chunk epilogue (att evac -> output projection -> normalize ->
    residual -> DMA) is software-pipelined into the next chunk's groups.
  - The scalar-engine exp stream (64 x [128,1024], ~71us) is the roofline;
    all PE work (~55us) and DVE work hide under it.
"""
import sys

sys.path.insert(0, "/opt/trn_rl_repo")

import ml_dtypes
import numpy as np

import concourse.bass as bass
import concourse.tile as tile
from concourse import bacc, mybir
from concourse.bass_utils import run_bass_kernel_spmd

F32 = mybir.dt.float32
BF16 = mybir.dt.bfloat16
FP8 = mybir.dt.float8e4

B, C, H, W = 4, 256, 64, 64
N = H * W          # 4096 tokens
NQ = N // 2        # 2048 query rows per core
D = C // 8         # 32 qk dim
G = 32             # groups
GS = C // G        # 8 channels per group
EPS = 1e-5
P = 128            # partitions
CT = C // P        # 2 channel tiles
CH = 512           # nq chunk
NCH = NQ // CH     # 4 chunks
MB = 128           # m block
NMB = N // MB      # 32 m blocks
NG2 = NMB // 2     # 16 groups of 2 m-blocks
SM_SCALE = float(D) ** -0.5
DR = mybir.MatmulPerfMode.DoubleRow

_CACHE = {}
_last_in_maps = None


def _build():
    if "nc" in _CACHE:
        return _CACHE["nc"]

    nc = bacc.Bacc("TRN2", target_bir_lowering=False, debug=False, num_devices=8)

    # x ships as fp8 already in the device layout [p, chunk, ct, CH]
    # (channel c = ct*128 + p); xc carries the bf16 residual correction
    # x - fp8(x) for the query half
    x8_ext = nc.declare_dram_parameter("x8", [P, N * CT], FP8, isOutput=False)
    xc_ext = nc.declare_dram_parameter("xc", [C, NQ], BF16, isOutput=False)
    wqt_ext = nc.declare_dram_parameter("wqt", [C, D], F32, isOutput=False)
    wkt_ext = nc.declare_dram_parameter("wkt", [C, D], F32, isOutput=False)
    wvt_ext = nc.declare_dram_parameter("wvt", [C, C], F32, isOutput=False)
    wpt_ext = nc.declare_dram_parameter("wpt", [C, C], F32, isOutput=False)
    # packed per-channel vectors: [gamma, beta, bv, bp]
    gbvp_ext = nc.declare_dram_parameter("gbvp", [C, 4], F32, isOutput=False)
    bqk_ext = nc.declare_dram_parameter("bqk", [D, 2], F32, isOutput=False)
    ind16_ext = nc.declare_dram_parameter("ind16", [P, G // CT], F32, isOutput=False)
    indb_ext = nc.declare_dram_parameter("indb", [G // CT, P], F32, isOutput=False)
    out_ext = nc.declare_dram_parameter("out", [C, NQ], F32, isOutput=True)

    GT = G // CT  # 16 groups per channel tile
    XP = N // 4   # x DMA piece size (overlap DMA with stats)

    with tile.TileContext(nc) as tc:
        with tc.tile_pool(name="const", bufs=1) as const, \
             tc.tile_pool(name="small", bufs=1) as small:
            dma_engs = [nc.sync, nc.gpsimd, nc.scalar]
            # tiny constants first — the GroupNorm combine matmuls need them
            # the moment the stats land
            ind16_sb = small.tile([P, GT], F32, tag="ind16")
            nc.sync.dma_start(out=ind16_sb, in_=ind16_ext[:])
            indb_sb = small.tile([GT, P], F32, tag="indb")
            nc.gpsimd.dma_start(out=indb_sb, in_=indb_ext[:])
            gbvp_sb = []
            for t in range(CT):
                tl = small.tile([P, 4], F32, tag=f"gbvp{t}", name=f"gbvp{t}")
                dma_engs[t].dma_start(out=tl, in_=gbvp_ext[t * P:(t + 1) * P, :])
                gbvp_sb.append(tl)
            bqk_sb = small.tile([D, 2], F32, tag="bqk")
            nc.scalar.dma_start(out=bqk_sb, in_=bqk_ext[:])
            gamma_sb = [gbvp_sb[t][:, 0:1] for t in range(CT)]
            beta_sb = [gbvp_sb[t][:, 1:2] for t in range(CT)]
            bv_sb = [gbvp_sb[t][:, 2:3] for t in range(CT)]
            bp_sb = [gbvp_sb[t][:, 3:4] for t in range(CT)]
            bq_sb = bqk_sb[:, 0:1]
            bk_sb = bqk_sb[:, 1:2]

            # x8 DMAs spread across the engine queues, one piece per
            # 512-token chunk so GroupNorm stats can chase the transfer
            x8 = const.tile([P, N // CH, CT, CH], FP8, tag="x8", name="x8")
            PCH = CT * CH  # flattened columns per chunk
            for pc in range(N // CH):
                qeng = dma_engs[pc % len(dma_engs)]
                qeng.dma_start(
                    out=x8[:, pc, :, :],
                    in_=x8_ext[:, pc * PCH:(pc + 1) * PCH].rearrange(
                        "p (t b) -> p t b", t=CT))

            # ---- persistent weight tiles ----
            wqt_sb, wkt_sb, wvt_sb, wpt_sb = [], [], [], []
            for t in range(CT):
                cs = slice(t * P, (t + 1) * P)
                w1 = const.tile([P, D], F32, tag=f"wqt{t}", name=f"wqt{t}")
                nc.gpsimd.dma_start(out=w1, in_=wqt_ext[cs, :])
                wqt_sb.append(w1)
                w2 = const.tile([P, D], F32, tag=f"wkt{t}", name=f"wkt{t}")
                nc.gpsimd.dma_start(out=w2, in_=wkt_ext[cs, :])
                wkt_sb.append(w2)
                # weights ride gpsimd: sync stays clear for the
                # latency-critical k_rep rearranges, scalar for the stats
                w3 = const.tile([P, C], F32, tag=f"wvt{t}", name=f"wvt{t}")
                nc.gpsimd.dma_start(out=w3, in_=wvt_ext[cs, :])
                wvt_sb.append(w3)
                w4 = const.tile([P, C], F32, tag=f"wpt{t}", name=f"wpt{t}")
                nc.gpsimd.dma_start(out=w4, in_=wpt_ext[cs, :])
                wpt_sb.append(w4)
            # bf16 copies for the (small) effective-bias matmuls — these only
            # need the raw weights, so cast as soon as the DMAs land
            wqt_hb = [const.tile([P, D], BF16, tag=f"wqthb{t}", name=f"wqthb{t}") for t in range(CT)]
            wkt_hb = [const.tile([P, D], BF16, tag=f"wkthb{t}", name=f"wkthb{t}") for t in range(CT)]
            wvt_hb = [const.tile([P, C], BF16, tag=f"wvthb{t}", name=f"wvthb{t}") for t in range(CT)]
            for t in range(CT):
                nc.vector.tensor_copy(out=wqt_hb[t], in_=wqt_sb[t])
                nc.vector.tensor_copy(out=wkt_hb[t], in_=wkt_sb[t])
                nc.vector.tensor_copy(out=wvt_hb[t], in_=wvt_sb[t])
            # residual correction tiles (DMAs emitted after the stats pass so
            # their issue slots don't delay the scalar engine's stats)
            xc_sb = [const.tile([P, NQ], BF16, tag=f"xc{t}", name=f"xc{t}") for t in range(CT)]
            ones8 = small.tile([P, 2, P], FP8, tag="ones8")
            nc.vector.memset(ones8, 1.0)
            eps_sb = small.tile([GT, 1], F32, tag="eps")
            nc.vector.memset(eps_sb, EPS)

            xqb = [const.tile([P, NQ], F32, tag=f"xqb{t}", name=f"xqb{t}") for t in range(CT)]
            scale_sb = [small.tile([P, 1], F32, tag=f"scale{t}", name=f"scale{t}") for t in range(CT)]
            shift_sb = [small.tile([P, 1], F32, tag=f"shift{t}", name=f"shift{t}") for t in range(CT)]

            # ---- GroupNorm stats overlapped with the x DMA ----
            # Both channel tiles go through ONE group-combine matmul pair to
            # minimize serial DVE<->PE<->Scalar ping-pong after the last stat.
            with tc.tile_pool(name="gn", bufs=2) as gn, \
                 tc.tile_pool(name="gnps", bufs=1, space="PSUM") as gnps:
                # stats split across engines: channel tile 0 on the DVE
                # (bn_stats), tile 1 on the otherwise-idle scalar engine
                # (accumulating sum / sum-of-squares passes) — neither alone
                # can keep pace with the x8 DMA
                mx_both = gn.tile([P, CT, 2], F32, tag="mxb")
                stats = gn.tile([P, N // CH, nc.vector.BN_STATS_DIM], F32, tag="st")
                part = gn.tile([P, 2, N // CH], F32, tag="part")
                for cn in range(N // CH):
                    nc.vector.bn_stats(out=stats[:, cn, :], in_=x8[:, cn, 0, :])
                    scr = gn.tile([P, CH], F32, tag="scr")
                    nc.scalar.activation(
                        out=scr, in_=x8[:, cn, 1, :],
                        func=mybir.ActivationFunctionType.Copy,
                        accum_out=part[:, 0, cn:cn + 1])
                    scr2 = gn.tile([P, CH], F32, tag="scr")
                    nc.scalar.activation(
                        out=scr2, in_=x8[:, cn, 1, :],
                        func=mybir.ActivationFunctionType.Square,
                        accum_out=part[:, 1, cn:cn + 1])
                nc.vector.bn_aggr(out=mx_both[:, 0, :], in_=stats)
                # in place: var -> E[x^2] = var + mean^2 (tile 0 only; the
                # scalar-side partials already sum x and x^2)
                nc.vector.scalar_tensor_tensor(
                    out=mx_both[:, 0, 1:2], in0=mx_both[:, 0, 0:1],
                    scalar=mx_both[:, 0, 0:1], in1=mx_both[:, 0, 1:2],
                    op0=mybir.AluOpType.mult, op1=mybir.AluOpType.add)
                tots = gn.tile([P, 2], F32, tag="tots")
                nc.vector.reduce_sum(out=tots, in_=part, axis=mybir.AxisListType.X)
                nc.vector.tensor_scalar_mul(
                    out=mx_both[:, 1, :], in0=tots, scalar1=1.0 / N)
                # xc transfers start only now — nothing latency-critical
                # rides the scalar queue behind them
                for t in range(CT):
                    nc.scalar.dma_start(
                        out=xc_sb[t], in_=xc_ext[t * P:(t + 1) * P, :])

                gps = gnps.tile([GT, CT, 2], F32, tag="gps")
                nc.tensor.matmul(
                    gps[:, :, :].rearrange("p a b -> p (a b)"), ind16_sb,
                    mx_both[:, :, :].rearrange("p a b -> p (a b)"),
                    start=True, stop=True)
                gsb = gn.tile([GT, CT, 2], F32, tag="gsb")
                nc.vector.tensor_copy(out=gsb, in_=gps)
                vneg = gn.tile([GT, CT, 1], F32, tag="vneg")
                nc.vector.tensor_mul(out=vneg, in0=gsb[:, :, 0:1], in1=gsb[:, :, 0:1])
                nc.vector.tensor_sub(out=vneg, in0=vneg, in1=gsb[:, :, 1:2])
                sd = gn.tile([GT, CT, 1], F32, tag="sd")
                nc.scalar.activation(
                    out=sd, in_=vneg,
                    func=mybir.ActivationFunctionType.Sqrt,
                    bias=eps_sb, scale=-1.0,
                )
                # dummy exp: pull the EXP activation-table load off the
                # first attention group's critical path
                dmy = gn.tile([GT, CT, 1], F32, tag="dmy")
                nc.scalar.activation(
                    out=dmy, in_=vneg,
                    func=mybir.ActivationFunctionType.Exp, scale=1.0)
                g2 = gn.tile([GT, CT, 2], F32, tag="g2")
                nc.vector.tensor_copy(out=g2[:, :, 0:1], in_=gsb[:, :, 0:1])
                nc.vector.reciprocal(out=g2[:, :, 1:2], in_=sd)

                bc = gnps.tile([P, CT, 2], F32, tag="bc")
                nc.tensor.matmul(
                    bc[:, :, :].rearrange("p a b -> p (a b)"), indb_sb,
                    g2[:, :, :].rearrange("p a b -> p (a b)"),
                    start=True, stop=True)
                for t in range(CT):
                    nc.vector.tensor_mul(out=scale_sb[t], in0=gamma_sb[t], in1=bc[:, t, 1:2])
                    sh1 = gn.tile([P, 1], F32, tag="sh1")
                    nc.vector.tensor_mul(out=sh1, in0=bc[:, t, 0:1], in1=scale_sb[t])
                    nc.vector.tensor_sub(out=shift_sb[t], in0=beta_sb[t], in1=sh1)

                # ---- scaled weights + effective biases ----
                # wqk8: [wq'|wk'|wq'|wk'] packed stationary (2x replicated),
                # fp8, both channel tiles stacked for DoubleRow
                wqk8 = const.tile([P, CT, 4 * D], FP8, tag="wqk8", name="wqk8")
                wvt8w = const.tile([P, CT, C], FP8, tag="wvt8w", name="wvt8w")
                wpt_h = [const.tile([P, C], BF16, tag=f"wpth{t}", name=f"wpth{t}") for t in range(CT)]
                for t in range(CT):
                    for j in range(2):
                        nc.vector.tensor_scalar_mul(
                            out=wqk8[:, t, (2 * j) * D:(2 * j + 1) * D],
                            in0=wqt_sb[t], scalar1=scale_sb[t])
                        nc.vector.tensor_scalar_mul(
                            out=wqk8[:, t, (2 * j + 1) * D:(2 * j + 2) * D],
                            in0=wkt_sb[t], scalar1=scale_sb[t])
                    nc.vector.tensor_scalar_mul(out=wvt8w[:, t, :], in0=wvt_sb[t], scalar1=scale_sb[t])
                    nc.vector.tensor_copy(out=wpt_h[t], in_=wpt_sb[t])
                shift_hb = [small.tile([P, 1], BF16, tag=f"shifthb{t}", name=f"shifthb{t}") for t in range(CT)]
                for t in range(CT):
                    nc.vector.tensor_copy(out=shift_hb[t], in_=shift_sb[t])

                with tc.tile_pool(name="bps", bufs=1, space="PSUM") as bps:
                    bq_eff = small.tile([D, 1], F32, tag="bqe")
                    bk_eff = small.tile([D, 1], F32, tag="bke")
                    psq = bps.tile([D, 1], F32, tag="pq")
                    psk = bps.tile([D, 1], F32, tag="pk")
                    for t in range(CT):
                        nc.tensor.matmul(psq, wqt_hb[t], shift_hb[t], start=(t == 0), stop=(t == CT - 1))
                        nc.tensor.matmul(psk, wkt_hb[t], shift_hb[t], start=(t == 0), stop=(t == CT - 1))
                    nc.vector.tensor_add(out=bq_eff, in0=psq, in1=bq_sb)
                    nc.vector.tensor_add(out=bk_eff, in0=psk, in1=bk_sb)
                    # interleaved bias vector [bq|bk|bq|bk] for the packed evac
                    qkbias = small.tile([P, 1], F32, tag="qkbias")
                    for j in range(2):
                        nc.vector.tensor_copy(out=qkbias[(2 * j) * D:(2 * j + 1) * D, :], in_=bq_eff)
                        nc.vector.tensor_copy(out=qkbias[(2 * j + 1) * D:(2 * j + 2) * D, :], in_=bk_eff)
                # bv/bp effective-bias SBUF tiles (chains emitted after qk)
                bv_eff = [small.tile([P, 1], F32, tag=f"bve{e}", name=f"bve{e}") for e in range(CT)]
                bv_eff_hb = [small.tile([P, 1], BF16, tag=f"bvehb{e}", name=f"bvehb{e}") for e in range(CT)]
                bp_effs = [small.tile([P, 1], F32, tag=f"bpe{f}", name=f"bpe{f}") for f in range(CT)]

            # ---- q/k (packed, 2x-replicated, fp8 DoubleRow) ----
            q_rep = const.tile([64, NQ], BF16, tag="qrep")
            k_rep = const.tile([64, N], BF16, tag="krep")
            vt8 = const.tile([P, NMB, C], FP8, tag="vt8")
            qkraw = const.tile([P, N], BF16, tag="qkraw")
            with tc.tile_pool(name="qkps", bufs=4, space="PSUM") as qkps:
                # DMA batches: chunk 0 and 1 rearranged immediately (S^T of
                # the first groups waits on them), the rest in wider batches
                batches = [(0, 1), (1, 2), (2, 4), (4, 6), (6, 8)]
                for lo, hi in batches:
                    for cn in range(lo, hi):
                        ns = slice(cn * CH, (cn + 1) * CH)
                        qkp = qkps.tile([P, CH], F32, tag="qkp", name=f"qkp{cn}")
                        nc.tensor.matmul(qkp, wqk8, x8[:, cn, :, :],
                                         start=True, stop=True, perf_mode=DR)
                        nc.vector.tensor_scalar_add(out=qkraw[:, ns], in0=qkp, scalar1=qkbias)
                    # partition rearrange: q bands {0-31,64-95}, k {32-63,96-127}
                    bs = slice(lo * CH, hi * CH)
                    nc.sync.dma_start(out=k_rep[0:32, bs], in_=qkraw[32:64, bs])
                    nc.sync.dma_start(out=k_rep[32:64, bs], in_=qkraw[96:128, bs])
                    if hi <= NQ // CH:
                        nc.gpsimd.dma_start(out=q_rep[0:32, bs], in_=qkraw[0:32, bs])
                        nc.gpsimd.dma_start(out=q_rep[32:64, bs], in_=qkraw[64:96, bs])

            # bv/bp effective-bias chains (bf16, off the qk critical path)
            with tc.tile_pool(name="bps2", bufs=1, space="PSUM") as bps2:
                for e in range(CT):
                    ps3 = bps2.tile([P, 1], F32, tag=f"pv{e}", name=f"psv{e}")
                    for t in range(CT):
                        nc.tensor.matmul(
                            ps3, wvt_hb[t][:, e * P:(e + 1) * P], shift_hb[t],
                            start=(t == 0), stop=(t == CT - 1),
                        )
                    nc.vector.tensor_add(out=bv_eff[e], in0=ps3, in1=bv_sb[e])
                    nc.vector.tensor_copy(out=bv_eff_hb[e], in_=bv_eff[e])
                for f in range(CT):
                    ps4 = bps2.tile([P, 1], F32, tag=f"pp{f}", name=f"psp{f}")
                    for e in range(CT):
                        nc.tensor.matmul(
                            ps4, wpt_h[e][:, f * P:(f + 1) * P], bv_eff_hb[e],
                            start=(e == 0), stop=(e == CT - 1),
                        )
                    nc.vector.tensor_add(out=bp_effs[f], in0=ps4, in1=bp_sb[f])

            # ---- attention ----
            with tc.tile_pool(name="stps", bufs=2, space="PSUM") as stps, \
                 tc.tile_pool(name="attps", bufs=1, space="PSUM") as attps, \
                 tc.tile_pool(name="rsps", bufs=1, space="PSUM") as rsps, \
                 tc.tile_pool(name="flex", bufs=1, space="PSUM") as flex, \
                 tc.tile_pool(name="pp", bufs=6) as pp, \
                 tc.tile_pool(name="attsb", bufs=4) as attsb, \
                 tc.tile_pool(name="osb", bufs=4) as osb, \
                 tc.tile_pool(name="rsb", bufs=2) as rsb:
                pend = None  # deferred epilogue payload of the previous chunk

                def eager_epilogue(ns_p, att2_p, rs_p):
                    """Emitted right at chunk end: frees the att2/rs PSUM
                    banks as fast as possible for the next chunk. The
                    DoubleRow ones-matmul already replicated the denominator
                    on every partition, so one approx-reciprocal pass gives
                    the full broadcast 1/den, and the softmax normalization
                    folds into the att2 evacuation itself."""
                    rec_bc = rsb.tile([P, CH], F32, tag="recbc")
                    nc.vector.reciprocal_approx_fast(out=rec_bc, in_=rs_p)
                    att_sb2 = attsb.tile([P, CT, CH], BF16, tag="attsb2")
                    for e in range(CT):
                        nc.vector.tensor_mul(
                            out=att_sb2[:, e, :], in0=att2_p[:, e, :], in1=rec_bc)
                    return (ns_p, att_sb2)

                def emit_epilogue_f(ep, f, final=False):
                    ns_p, att_sb2 = ep
                    fs = slice(f * P, (f + 1) * P)
                    pjt = flex.tile([P, 2, CH // 2], F32, tag="flex", name=f"pj{f}")
                    pj = pjt[:, :, :].rearrange("p a b -> p (a b)")
                    for e in range(CT):
                        nc.tensor.matmul(
                            pj, wpt_h[e][:, f * P:(f + 1) * P],
                            att_sb2[:, e, :],
                            start=(e == 0), stop=(e == CT - 1),
                        )
                    o = osb.tile([P, CH], F32, tag="o")
                    nc.vector.tensor_add(out=o, in0=pj, in1=xqb[f][:, ns_p])
                    # split each strip across queues so the tail drains fast
                    # (keep gpsimd out of the tail: its queue drain gates
                    # kernel teardown)
                    hc = CH // 2
                    oeng = [nc.sync, nc.scalar] if final else [nc.sync, nc.gpsimd]
                    ne = len(oeng)
                    for hh in range(2):
                        cs2 = slice(ns_p.start + hh * hc, ns_p.start + (hh + 1) * hc)
                        oeng[(2 * f + hh) % ne].dma_start(
                            out=out_ext[fs, cs2], in_=o[:, hh * hc:(hh + 1) * hc])

                # flattened (chunk, group) stream: the S^T/exp front runs LAG
                # groups ahead of the rs/PV drain, ACROSS chunk boundaries,
                # so the PE never sits behind the trailing PV of a chunk
                LAG = 2
                TOT = NCH * NG2
                att2_t = [None] * NCH
                rs_t = [None] * NCH
                p_tiles = [None] * TOT
                for k in range(TOT + LAG):
                    if k < TOT:
                        ch, g = divmod(k, NG2)
                        ns = slice(ch * CH, (ch + 1) * CH)
                        if g == 0:
                            att2_t[ch] = attps.tile([P, CT, CH], F32, tag="att2", name=f"att2c{ch}")
                            rs_t[ch] = rsps.tile([P, CH], F32, tag="rs", name=f"rsc{ch}")
                        # 2 row-packed S^T matmuls
                        stg = stps.tile([P, 2, CH], F32, tag="stg")
                        for j in range(2):
                            mb = g * 2 + j
                            nc.tensor.matmul(
                                stg[:, j, :],
                                k_rep[32 * j:32 * (j + 1), mb * MB:(mb + 1) * MB],
                                q_rep[32 * j:32 * (j + 1), ns],
                                start=True, stop=True,
                                tile_position=(32 * j, 0),
                            )
                        if ch == 0:
                            # v^T production rides inside chunk 0, two
                            # m-blocks per 1-bank flex granule (fp8 DoubleRow
                            # over both channel tiles)
                            vpt = flex.tile([P, 2, CH // 2], F32, tag="flex", name=f"vp{g}")
                            for i in range(2):
                                mb = 2 * g + i
                                nc.tensor.matmul(
                                    vpt[:, i, :],
                                    x8[:, mb // 4, :, (mb % 4) * MB:(mb % 4 + 1) * MB],
                                    wvt8w,
                                    start=True, stop=True, perf_mode=DR,
                                )
                            nc.vector.tensor_copy(
                                out=vt8[:, 2 * g:2 * g + 2, :], in_=vpt)
                        if ch == 1 and g < CT:
                            # residual base: fp8 x + bf16 correction +
                            # projection bias (DVE has slack here)
                            f = g
                            nq_ch = NQ // CH
                            nc.vector.scalar_tensor_tensor(
                                out=xqb[f][:, :].rearrange("p (a b) -> p a b", a=nq_ch),
                                in0=x8[:, 0:nq_ch, f, :], scalar=bp_effs[f],
                                in1=xc_sb[f][:, :].rearrange("p (a b) -> p a b", a=nq_ch),
                                op0=mybir.AluOpType.add, op1=mybir.AluOpType.add)
                        pg = pp.tile([P, 2, CH], FP8, tag="pg")
                        nc.scalar.activation(
                            out=pg, in_=stg,
                            func=mybir.ActivationFunctionType.Exp,
                            scale=SM_SCALE,
                        )
                        p_tiles[k] = pg
                        # deferred epilogue of the previous chunk, one output
                        # strip per group so it never swamps one group's slack
                        if g in (4, 5) and pend is not None:
                            emit_epilogue_f(pend, g - 4)
                            if g == 5:
                                pend = None
                    if k >= LAG:
                        kp = k - LAG
                        chp, gp = divmod(kp, NG2)
                        pg = p_tiles[kp]
                        p_tiles[kp] = None
                        # denominator: fp8 DoubleRow ones-matmul, result
                        # replicated across all 128 partitions
                        nc.tensor.matmul(
                            rs_t[chp], ones8, pg,
                            start=(gp == 0), stop=(gp == NG2 - 1),
                            perf_mode=DR,
                        )
                        # P*V: fp8 DoubleRow, two m-blocks per pass
                        for e in range(CT):
                            nc.tensor.matmul(
                                att2_t[chp][:, e, :],
                                vt8[:, 2 * gp:2 * gp + 2, e * P:(e + 1) * P],
                                pg,
                                start=(gp == 0), stop=(gp == NG2 - 1),
                                perf_mode=DR,
                            )
                        if gp == NG2 - 1:
                            nsp = slice(chp * CH, (chp + 1) * CH)
                            pend = eager_epilogue(nsp, att2_t[chp], rs_t[chp])
                for f in range(CT):
                    emit_epilogue_f(pend, f, final=True)

    nc.compile()
    _CACHE["nc"] = nc
    return nc


def qk_query_dma(cn):
    """If x-chunk cn lies in the query half (columns 0:NQ), return its
    local chunk index."""
    return cn if cn < NQ // CH else None


def kernel(x, gamma, beta, wq, bq, wk, bk, wv, bv, wp, bp):
    x = np.ascontiguousarray(np.asarray(x, dtype=np.float32))
    nc = _build()

    GT = G // CT
    ind16 = np.zeros((P, GT), np.float32)
    for c in range(P):
        ind16[c, c // GS] = 1.0 / GS
    indb = np.zeros((GT, P), np.float32)
    for c in range(P):
        indb[c // GS, c] = 1.0

    common = {
        "wqt": np.ascontiguousarray(np.asarray(wq, np.float32).T),
        "wkt": np.ascontiguousarray(np.asarray(wk, np.float32).T),
        "wvt": np.ascontiguousarray(np.asarray(wv, np.float32).T),
        "wpt": np.ascontiguousarray(np.asarray(wp, np.float32).T),
        "gbvp": np.ascontiguousarray(np.stack(
            [np.asarray(gamma, np.float32), np.asarray(beta, np.float32),
             np.asarray(bv, np.float32), np.asarray(bp, np.float32)], axis=1)),
        "bqk": np.ascontiguousarray(np.stack(
            [np.asarray(bq, np.float32), np.asarray(bk, np.float32)], axis=1)),
        "ind16": ind16,
        "indb": indb,
    }

    xf = x.reshape(B, C, N)
    x8all = xf.astype(ml_dtypes.float8_e4m3)
    # bf16 correction x - fp8(x), exact residual reconstruction on device
    xcall = (xf - x8all.astype(np.float32)).astype(ml_dtypes.bfloat16)
    in_maps = []
    for core in range(8):
        b, half = core // 2, core % 2
        m = dict(common)
        # put this core's query tokens in columns 0:NQ (token order within
        # the key axis is irrelevant to GroupNorm stats and softmax sums)
        hs = slice(half * NQ, (half + 1) * NQ)
        if half == 0:
            xp8 = x8all[b]
        else:
            xp8 = np.concatenate([x8all[b][:, NQ:], x8all[b][:, :NQ]], axis=1)
        # device layout [p, chunk, ct, CH] with channel c = ct*128 + p
        m["x8"] = np.ascontiguousarray(
            xp8.reshape(CT, P, N // CH, CH).transpose(1, 2, 0, 3).reshape(P, N * CT))
        m["xc"] = np.ascontiguousarray(xcall[b][:, hs])
        in_maps.append(m)

    global _last_in_maps
    _last_in_maps = in_maps
    res = run_bass_kernel_spmd(nc, in_maps, list(range(8)))

    y = np.empty((B, C, N), np.float32)
    for core in range(8):
        b, half = core // 2, core % 2
        y[b][:, half * NQ:(half + 1) * NQ] = res.results[core]["out"]
    return y.reshape(B, C, H, W)


# revision 75
# speedup vs baseline: 1.0142x; 1.0142x over previous
"""AttentionBlock (GroupNorm + 1x1-conv QKV self-attention + residual) on 8 TRN2 cores.

Sharding: data-parallel over batch B=4 x sequence-parallel over the 4096
tokens (2 cores per batch element, each handling 2048 query rows; K/V and
GroupNorm are computed redundantly per core pair — they are cheap relative
to attention).

Per-core device kernel (fp8 attention matmuls, GN stats + residual fp32/bf16):
  - x ships as bf16 only (no fp32 copy): matmul/stats operand AND the
    residual base (bf16 residual error ~1e-3 of output absmax, well under
    tolerance). GroupNorm stats overlap the input DMA (bn_stats per piece,
    group-combine via tiny indicator matmuls on the PE).
  - GroupNorm folds into the QKV weights: h = scale_c*x + shift_c, so q/k/v
    come straight from x with per-channel-scaled weights + effective biases.
  - q and k are computed together, 2x-replicated, by a single packed
    stationary [wq|wk|wq|wk] [128,128] matmul per 512-token chunk (one
    moving-data pass instead of four), evacuated once with the interleaved
    bias vector, then partition-rearranged into q_rep/k_rep [64, *] via
    SBUF->SBUF DMAs (free on the DMA engines).
  - S^T: 2x row-packed bf16 matmuls (K=32 contraction) into a 2-bank PSUM
    tile; one [128,1024] exp on the scalar engine writes fp8e4 directly
    (softmax scale fused; S range ~±0.8 so no max subtraction).
  - P*V runs in fp8 DoubleRow mode: each matmul contracts TWO 128-key
    m-blocks per pass (2x PE throughput); V^T is evacuated to fp8.
  - Softmax denominator: one fp8 DoubleRow ones-matmul [128,2,128] per
    group accumulates the column sums replicated across ALL 128 PSUM
    partitions; the epilogue takes a [1,512] reciprocal (tiny) and
    re-broadcasts it into the same PSUM bank with a contraction-1 matmul.
    No mask, no GpSimd partition reduce, no [128,512] reciprocal.
  - The per-chunk epilogue (att evac -> output projection -> normalize ->
    residual -> DMA) is software-pipelined into the next chunk's groups.
  - The scalar-engine exp stream (64 x [128,1024], ~71us) is the roofline;
    all PE work (~55us) and DVE work hide under it.
"""
import sys

sys.path.insert(0, "/opt/trn_rl_repo")

import ml_dtypes
import numpy as np

import concourse.bass as bass
import concourse.tile as tile
from concourse import bacc, mybir
from concourse.bass_utils import run_bass_kernel_spmd

F32 = mybir.dt.float32
BF16 = mybir.dt.bfloat16
FP8 = mybir.dt.float8e4

B, C, H, W = 4, 256, 64, 64
N = H * W          # 4096 tokens
NQ = N // 2        # 2048 query rows per core
D = C // 8         # 32 qk dim
G = 32             # groups
GS = C // G        # 8 channels per group
EPS = 1e-5
P = 128            # partitions
CT = C // P        # 2 channel tiles
CH = 512           # nq chunk
NCH = NQ // CH     # 4 chunks
MB = 128           # m block
NMB = N // MB      # 32 m blocks
NG2 = NMB // 2     # 16 groups of 2 m-blocks
SM_SCALE = float(D) ** -0.5
DR = mybir.MatmulPerfMode.DoubleRow

_CACHE = {}
_last_in_maps = None


def _build():
    if "nc" in _CACHE:
        return _CACHE["nc"]

    nc = bacc.Bacc("TRN2", target_bir_lowering=False, debug=False, num_devices=8)

    # x ships as fp8 already in the device layout [p, chunk, ct, CH]
    # (channel c = ct*128 + p); xc carries the bf16 residual correction
    # x - fp8(x) for the query half
    x8_ext = nc.declare_dram_parameter("x8", [P, N * CT], FP8, isOutput=False)
    xc_ext = nc.declare_dram_parameter("xc", [C, NQ], BF16, isOutput=False)
    wqt_ext = nc.declare_dram_parameter("wqt", [C, D], F32, isOutput=False)
    wkt_ext = nc.declare_dram_parameter("wkt", [C, D], F32, isOutput=False)
    wvt_ext = nc.declare_dram_parameter("wvt", [C, C], F32, isOutput=False)
    wpt_ext = nc.declare_dram_parameter("wpt", [C, C], F32, isOutput=False)
    # packed per-channel vectors: [gamma, beta, bv, bp]
    gbvp_ext = nc.declare_dram_parameter("gbvp", [C, 4], F32, isOutput=False)
    bqk_ext = nc.declare_dram_parameter("bqk", [D, 2], F32, isOutput=False)
    ind16_ext = nc.declare_dram_parameter("ind16", [P, G // CT], F32, isOutput=False)
    indb_ext = nc.declare_dram_parameter("indb", [G // CT, P], F32, isOutput=False)
    out_ext = nc.declare_dram_parameter("out", [C, NQ], F32, isOutput=True)

    GT = G // CT  # 16 groups per channel tile
    XP = N // 4   # x DMA piece size (overlap DMA with stats)

    with tile.TileContext(nc) as tc:
        with tc.tile_pool(name="const", bufs=1) as const, \
             tc.tile_pool(name="small", bufs=1) as small:
            dma_engs = [nc.sync, nc.gpsimd, nc.scalar]
            # tiny constants first — the GroupNorm combine matmuls need them
            # the moment the stats land
            ind16_sb = small.tile([P, GT], F32, tag="ind16")
            nc.sync.dma_start(out=ind16_sb, in_=ind16_ext[:])
            indb_sb = small.tile([GT, P], F32, tag="indb")
            nc.gpsimd.dma_start(out=indb_sb, in_=indb_ext[:])
            gbvp_sb = []
            for t in range(CT):
                tl = small.tile([P, 4], F32, tag=f"gbvp{t}", name=f"gbvp{t}")
                dma_engs[t].dma_start(out=tl, in_=gbvp_ext[t * P:(t + 1) * P, :])
                gbvp_sb.append(tl)
            bqk_sb = small.tile([D, 2], F32, tag="bqk")
            nc.scalar.dma_start(out=bqk_sb, in_=bqk_ext[:])
            gamma_sb = [gbvp_sb[t][:, 0:1] for t in range(CT)]
            beta_sb = [gbvp_sb[t][:, 1:2] for t in range(CT)]
            bv_sb = [gbvp_sb[t][:, 2:3] for t in range(CT)]
            bp_sb = [gbvp_sb[t][:, 3:4] for t in range(CT)]
            bq_sb = bqk_sb[:, 0:1]
            bk_sb = bqk_sb[:, 1:2]

            # x8 DMAs spread across the engine queues, one piece per
            # 512-token chunk so GroupNorm stats can chase the transfer
            x8 = const.tile([P, N // CH, CT, CH], FP8, tag="x8", name="x8")
            PCH = CT * CH  # flattened columns per chunk
            for pc in range(N // CH):
                qeng = dma_engs[pc % len(dma_engs)]
                qeng.dma_start(
                    out=x8[:, pc, :, :],
                    in_=x8_ext[:, pc * PCH:(pc + 1) * PCH].rearrange(
                        "p (t b) -> p t b", t=CT))

            # ---- persistent weight tiles ----
            wqt_sb, wkt_sb, wvt_sb, wpt_sb = [], [], [], []
            for t in range(CT):
                cs = slice(t * P, (t + 1) * P)
                w1 = const.tile([P, D], F32, tag=f"wqt{t}", name=f"wqt{t}")
                nc.gpsimd.dma_start(out=w1, in_=wqt_ext[cs, :])
                wqt_sb.append(w1)
                w2 = const.tile([P, D], F32, tag=f"wkt{t}", name=f"wkt{t}")
                nc.gpsimd.dma_start(out=w2, in_=wkt_ext[cs, :])
                wkt_sb.append(w2)
                # the big weight transfers are emitted after the stats loop
                # on the scalar queue, so neither rearrange queue (sync=k,
                # gpsimd=q) ever has a bulk transfer in front of it
                w3 = const.tile([P, C], F32, tag=f"wvt{t}", name=f"wvt{t}")
                wvt_sb.append(w3)
                w4 = const.tile([P, C], F32, tag=f"wpt{t}", name=f"wpt{t}")
                wpt_sb.append(w4)
            # bf16 copies for the (small) effective-bias matmuls (wvt_hb is
            # cast later, once the deferred wvt transfer has landed)
            wqt_hb = [const.tile([P, D], BF16, tag=f"wqthb{t}", name=f"wqthb{t}") for t in range(CT)]
            wkt_hb = [const.tile([P, D], BF16, tag=f"wkthb{t}", name=f"wkthb{t}") for t in range(CT)]
            wvt_hb = [const.tile([P, C], BF16, tag=f"wvthb{t}", name=f"wvthb{t}") for t in range(CT)]
            for t in range(CT):
                nc.vector.tensor_copy(out=wqt_hb[t], in_=wqt_sb[t])
                nc.vector.tensor_copy(out=wkt_hb[t], in_=wkt_sb[t])
            # residual correction tiles (DMAs emitted after the stats pass so
            # their issue slots don't delay the scalar engine's stats)
            xc_sb = [const.tile([P, NQ], BF16, tag=f"xc{t}", name=f"xc{t}") for t in range(CT)]
            ones8 = small.tile([P, 2, P], FP8, tag="ones8")
            nc.vector.memset(ones8, 1.0)
            eps_sb = small.tile([GT, 1], F32, tag="eps")
            nc.vector.memset(eps_sb, EPS)

            xqb = [const.tile([P, NQ], F32, tag=f"xqb{t}", name=f"xqb{t}") for t in range(CT)]
            scale_sb = [small.tile([P, 1], F32, tag=f"scale{t}", name=f"scale{t}") for t in range(CT)]
            shift_sb = [small.tile([P, 1], F32, tag=f"shift{t}", name=f"shift{t}") for t in range(CT)]

            # ---- GroupNorm stats overlapped with the x DMA ----
            # Both channel tiles go through ONE group-combine matmul pair to
            # minimize serial DVE<->PE<->Scalar ping-pong after the last stat.
            with tc.tile_pool(name="gn", bufs=2) as gn, \
                 tc.tile_pool(name="gnps", bufs=1, space="PSUM") as gnps:
                # stats split across engines: the DVE (bn_stats) takes tile 0
                # plus the late half of tile 1; the slower scalar-accumulate
                # path takes tile 1's first four pieces — neither engine alone
                # keeps pace with the x8 DMA
                NPC = N // CH
                SCN = 4  # pieces handled by the scalar engine
                mx_both = gn.tile([P, CT, 2], F32, tag="mxb")
                stats = gn.tile([P, NPC, nc.vector.BN_STATS_DIM], F32, tag="st")
                stat1 = gn.tile([P, NPC - SCN, nc.vector.BN_STATS_DIM], F32, tag="st1")
                part = gn.tile([P, 2, SCN], F32, tag="part")
                for cn in range(NPC):
                    if cn < SCN:
                        scr = gn.tile([P, CH], F32, tag="scr")
                        nc.scalar.activation(
                            out=scr, in_=x8[:, cn, 1, :],
                            func=mybir.ActivationFunctionType.Copy,
                            accum_out=part[:, 0, cn:cn + 1])
                        scr2 = gn.tile([P, CH], F32, tag="scr")
                        nc.scalar.activation(
                            out=scr2, in_=x8[:, cn, 1, :],
                            func=mybir.ActivationFunctionType.Square,
                            accum_out=part[:, 1, cn:cn + 1])
                    nc.vector.bn_stats(out=stats[:, cn, :], in_=x8[:, cn, 0, :])
                    if cn >= SCN:
                        nc.vector.bn_stats(
                            out=stat1[:, cn - SCN, :], in_=x8[:, cn, 1, :])
                # big non-critical transfers start only now: nothing
                # latency-critical rides the scalar queue behind them
                for t in range(CT):
                    nc.scalar.dma_start(
                        out=wvt_sb[t], in_=wvt_ext[t * P:(t + 1) * P, :])
                    nc.scalar.dma_start(
                        out=wpt_sb[t], in_=wpt_ext[t * P:(t + 1) * P, :])
                    nc.scalar.dma_start(
                        out=xc_sb[t], in_=xc_ext[t * P:(t + 1) * P, :])
                nc.vector.bn_aggr(out=mx_both[:, 0, :], in_=stats)
                # in place: var -> E[x^2] = var + mean^2
                nc.vector.scalar_tensor_tensor(
                    out=mx_both[:, 0, 1:2], in0=mx_both[:, 0, 0:1],
                    scalar=mx_both[:, 0, 0:1], in1=mx_both[:, 0, 1:2],
                    op0=mybir.AluOpType.mult, op1=mybir.AluOpType.add)
                # tile 1: combine the DVE half (mean/var of the back pieces)
                # with the scalar partial sums of the front pieces
                mv1 = gn.tile([P, 2], F32, tag="mv1")
                nc.vector.bn_aggr(out=mv1, in_=stat1)
                nc.vector.scalar_tensor_tensor(
                    out=mv1[:, 1:2], in0=mv1[:, 0:1], scalar=mv1[:, 0:1],
                    in1=mv1[:, 1:2],
                    op0=mybir.AluOpType.mult, op1=mybir.AluOpType.add)
                tots = gn.tile([P, 2], F32, tag="tots")
                nc.vector.reduce_sum(out=tots, in_=part, axis=mybir.AxisListType.X)
                wD = (NPC - SCN) / NPC   # weight of the DVE half
                wS = 1.0 / (SCN * CH) * (SCN / NPC)  # partial-sum scale
                nc.vector.tensor_scalar_mul(
                    out=mx_both[:, 1, :], in0=mv1, scalar1=wD)
                nc.vector.scalar_tensor_tensor(
                    out=mx_both[:, 1, :], in0=tots, scalar=wS,
                    in1=mx_both[:, 1, :],
                    op0=mybir.AluOpType.mult, op1=mybir.AluOpType.add)

                gps = gnps.tile([GT, CT, 2], F32, tag="gps")
                nc.tensor.matmul(
                    gps[:, :, :].rearrange("p a b -> p (a b)"), ind16_sb,
                    mx_both[:, :, :].rearrange("p a b -> p (a b)"),
                    start=True, stop=True)
                gsb = gn.tile([GT, CT, 2], F32, tag="gsb")
                nc.vector.tensor_copy(out=gsb, in_=gps)
                vneg = gn.tile([GT, CT, 1], F32, tag="vneg")
                nc.vector.tensor_mul(out=vneg, in0=gsb[:, :, 0:1], in1=gsb[:, :, 0:1])
                nc.vector.tensor_sub(out=vneg, in0=vneg, in1=gsb[:, :, 1:2])
                sd = gn.tile([GT, CT, 1], F32, tag="sd")
                nc.scalar.activation(
                    out=sd, in_=vneg,
                    func=mybir.ActivationFunctionType.Sqrt,
                    bias=eps_sb, scale=-1.0,
                )
                # dummy exp: pull the EXP activation-table load off the
                # first attention group's critical path
                dmy = gn.tile([GT, CT, 1], F32, tag="dmy")
                nc.scalar.activation(
                    out=dmy, in_=vneg,
                    func=mybir.ActivationFunctionType.Exp, scale=1.0)
                g2 = gn.tile([GT, CT, 2], F32, tag="g2")
                nc.vector.tensor_copy(out=g2[:, :, 0:1], in_=gsb[:, :, 0:1])
                nc.vector.reciprocal(out=g2[:, :, 1:2], in_=sd)

                bc = gnps.tile([P, CT, 2], F32, tag="bc")
                nc.tensor.matmul(
                    bc[:, :, :].rearrange("p a b -> p (a b)"), indb_sb,
                    g2[:, :, :].rearrange("p a b -> p (a b)"),
                    start=True, stop=True)
                for t in range(CT):
                    nc.vector.tensor_mul(out=scale_sb[t], in0=gamma_sb[t], in1=bc[:, t, 1:2])
                    sh1 = gn.tile([P, 1], F32, tag="sh1")
                    nc.vector.tensor_mul(out=sh1, in0=bc[:, t, 0:1], in1=scale_sb[t])
                    nc.vector.tensor_sub(out=shift_sb[t], in0=beta_sb[t], in1=sh1)

                # ---- scaled weights + effective biases ----
                # wqk8: [wq'|wk'|wq'|wk'] packed stationary (2x replicated),
                # fp8, both channel tiles stacked for DoubleRow
                wqk8 = const.tile([P, CT, 4 * D], FP8, tag="wqk8", name="wqk8")
                wvt8w = const.tile([P, CT, C], FP8, tag="wvt8w", name="wvt8w")
                wpt_h = [const.tile([P, C], BF16, tag=f"wpth{t}", name=f"wpth{t}") for t in range(CT)]
                for t in range(CT):
                    for j in range(2):
                        nc.vector.tensor_scalar_mul(
                            out=wqk8[:, t, (2 * j) * D:(2 * j + 1) * D],
                            in0=wqt_sb[t], scalar1=scale_sb[t])
                        nc.vector.tensor_scalar_mul(
                            out=wqk8[:, t, (2 * j + 1) * D:(2 * j + 2) * D],
                            in0=wkt_sb[t], scalar1=scale_sb[t])
                    nc.vector.tensor_scalar_mul(out=wvt8w[:, t, :], in0=wvt_sb[t], scalar1=scale_sb[t])
                    nc.vector.tensor_copy(out=wpt_h[t], in_=wpt_sb[t])
                shift_hb = [small.tile([P, 1], BF16, tag=f"shifthb{t}", name=f"shifthb{t}") for t in range(CT)]
                for t in range(CT):
                    nc.vector.tensor_copy(out=shift_hb[t], in_=shift_sb[t])
                    nc.vector.tensor_copy(out=wvt_hb[t], in_=wvt_sb[t])

                with tc.tile_pool(name="bps", bufs=1, space="PSUM") as bps:
                    bq_eff = small.tile([D, 1], F32, tag="bqe")
                    bk_eff = small.tile([D, 1], F32, tag="bke")
                    psq = bps.tile([D, 1], F32, tag="pq")
                    psk = bps.tile([D, 1], F32, tag="pk")
                    for t in range(CT):
                        nc.tensor.matmul(psq, wqt_hb[t], shift_hb[t], start=(t == 0), stop=(t == CT - 1))
                        nc.tensor.matmul(psk, wkt_hb[t], shift_hb[t], start=(t == 0), stop=(t == CT - 1))
                    nc.vector.tensor_add(out=bq_eff, in0=psq, in1=bq_sb)
                    nc.vector.tensor_add(out=bk_eff, in0=psk, in1=bk_sb)
                    # interleaved bias vector [bq|bk|bq|bk] for the packed evac
                    qkbias = small.tile([P, 1], F32, tag="qkbias")
                    for j in range(2):
                        nc.vector.tensor_copy(out=qkbias[(2 * j) * D:(2 * j + 1) * D, :], in_=bq_eff)
                        nc.vector.tensor_copy(out=qkbias[(2 * j + 1) * D:(2 * j + 2) * D, :], in_=bk_eff)
                # bv/bp effective-bias SBUF tiles (chains emitted after qk)
                bv_eff = [small.tile([P, 1], F32, tag=f"bve{e}", name=f"bve{e}") for e in range(CT)]
                bv_eff_hb = [small.tile([P, 1], BF16, tag=f"bvehb{e}", name=f"bvehb{e}") for e in range(CT)]
                bp_effs = [small.tile([P, 1], F32, tag=f"bpe{f}", name=f"bpe{f}") for f in range(CT)]

            # ---- q/k (packed, 2x-replicated, fp8 DoubleRow) ----
            q_rep = const.tile([64, NQ], BF16, tag="qrep")
            k_rep = const.tile([64, N], BF16, tag="krep")
            vt8 = const.tile([P, NMB, C], FP8, tag="vt8")
            qkraw = const.tile([P, N], BF16, tag="qkraw")
            with tc.tile_pool(name="qkps", bufs=4, space="PSUM") as qkps:
                # DMA batches: chunk 0 and 1 rearranged immediately (S^T of
                # the first groups waits on them), the rest in wider batches
                batches = [(0, 1), (1, 2), (2, 4), (4, 6), (6, 8)]
                for lo, hi in batches:
                    for cn in range(lo, hi):
                        ns = slice(cn * CH, (cn + 1) * CH)
                        qkp = qkps.tile([P, CH], F32, tag="qkp", name=f"qkp{cn}")
                        nc.tensor.matmul(qkp, wqk8, x8[:, cn, :, :],
                                         start=True, stop=True, perf_mode=DR)
                        nc.vector.tensor_scalar_add(out=qkraw[:, ns], in0=qkp, scalar1=qkbias)
                    # partition rearrange: q bands {0-31,64-95}, k {32-63,96-127}
                    bs = slice(lo * CH, hi * CH)
                    nc.sync.dma_start(out=k_rep[0:32, bs], in_=qkraw[32:64, bs])
                    nc.sync.dma_start(out=k_rep[32:64, bs], in_=qkraw[96:128, bs])
                    if hi <= NQ // CH:
                        nc.gpsimd.dma_start(out=q_rep[0:32, bs], in_=qkraw[0:32, bs])
                        nc.gpsimd.dma_start(out=q_rep[32:64, bs], in_=qkraw[64:96, bs])

            # bv/bp effective-bias chains (bf16, off the qk critical path)
            with tc.tile_pool(name="bps2", bufs=1, space="PSUM") as bps2:
                for e in range(CT):
                    ps3 = bps2.tile([P, 1], F32, tag=f"pv{e}", name=f"psv{e}")
                    for t in range(CT):
                        nc.tensor.matmul(
                            ps3, wvt_hb[t][:, e * P:(e + 1) * P], shift_hb[t],
                            start=(t == 0), stop=(t == CT - 1),
                        )
                    nc.vector.tensor_add(out=bv_eff[e], in0=ps3, in1=bv_sb[e])
                    nc.vector.tensor_copy(out=bv_eff_hb[e], in_=bv_eff[e])
                for f in range(CT):
                    ps4 = bps2.tile([P, 1], F32, tag=f"pp{f}", name=f"psp{f}")
                    for e in range(CT):
                        nc.tensor.matmul(
                            ps4, wpt_h[e][:, f * P:(f + 1) * P], bv_eff_hb[e],
                            start=(e == 0), stop=(e == CT - 1),
                        )
                    nc.vector.tensor_add(out=bp_effs[f], in0=ps4, in1=bp_sb[f])

            # ---- attention ----
            with tc.tile_pool(name="stps", bufs=2, space="PSUM") as stps, \
                 tc.tile_pool(name="attps", bufs=1, space="PSUM") as attps, \
                 tc.tile_pool(name="rsps", bufs=1, space="PSUM") as rsps, \
                 tc.tile_pool(name="flex", bufs=1, space="PSUM") as flex, \
                 tc.tile_pool(name="pp", bufs=6) as pp, \
                 tc.tile_pool(name="attsb", bufs=4) as attsb, \
                 tc.tile_pool(name="osb", bufs=4) as osb, \
                 tc.tile_pool(name="rsb", bufs=2) as rsb:
                pend = None  # deferred epilogue payload of the previous chunk

                def eager_epilogue(ns_p, att2_p, rs_p):
                    """Emitted right at chunk end: frees the att2/rs PSUM
                    banks as fast as possible for the next chunk. The
                    DoubleRow ones-matmul already replicated the denominator
                    on every partition, so one approx-reciprocal pass gives
                    the full broadcast 1/den, and the softmax normalization
                    folds into the att2 evacuation itself."""
                    rec_bc = rsb.tile([P, CH], F32, tag="recbc")
                    nc.vector.reciprocal_approx_fast(out=rec_bc, in_=rs_p)
                    att_sb2 = attsb.tile([P, CT, CH], BF16, tag="attsb2")
                    for e in range(CT):
                        nc.vector.tensor_mul(
                            out=att_sb2[:, e, :], in0=att2_p[:, e, :], in1=rec_bc)
                    return (ns_p, att_sb2)

                def emit_epilogue_f(ep, f, final=False):
                    ns_p, att_sb2 = ep
                    fs = slice(f * P, (f + 1) * P)
                    pjt = flex.tile([P, 2, CH // 2], F32, tag="flex", name=f"pj{f}")
                    pj = pjt[:, :, :].rearrange("p a b -> p (a b)")
                    for e in range(CT):
                        nc.tensor.matmul(
                            pj, wpt_h[e][:, f * P:(f + 1) * P],
                            att_sb2[:, e, :],
                            start=(e == 0), stop=(e == CT - 1),
                        )
                    o = osb.tile([P, CH], F32, tag="o")
                    nc.vector.tensor_add(out=o, in0=pj, in1=xqb[f][:, ns_p])
                    # split each strip across queues so the tail drains fast
                    # (keep gpsimd out of the tail: its queue drain gates
                    # kernel teardown)
                    hc = CH // 2
                    oeng = [nc.sync, nc.scalar] if final else [nc.sync, nc.gpsimd]
                    ne = len(oeng)
                    for hh in range(2):
                        cs2 = slice(ns_p.start + hh * hc, ns_p.start + (hh + 1) * hc)
                        oeng[(2 * f + hh) % ne].dma_start(
                            out=out_ext[fs, cs2], in_=o[:, hh * hc:(hh + 1) * hc])

                # flattened (chunk, group) stream: the S^T/exp front runs LAG
                # groups ahead of the rs/PV drain, ACROSS chunk boundaries,
                # so the PE never sits behind the trailing PV of a chunk
                LAG = 2
                TOT = NCH * NG2
                att2_t = [None] * NCH
                rs_t = [None] * NCH
                p_tiles = [None] * TOT
                for k in range(TOT + LAG):
                    if k < TOT:
                        ch, g = divmod(k, NG2)
                        ns = slice(ch * CH, (ch + 1) * CH)
                        if g == 0:
                            att2_t[ch] = attps.tile([P, CT, CH], F32, tag="att2", name=f"att2c{ch}")
                            rs_t[ch] = rsps.tile([P, CH], F32, tag="rs", name=f"rsc{ch}")
                        # 2 row-packed S^T matmuls
                        stg = stps.tile([P, 2, CH], F32, tag="stg")
                        for j in range(2):
                            mb = g * 2 + j
                            nc.tensor.matmul(
                                stg[:, j, :],
                                k_rep[32 * j:32 * (j + 1), mb * MB:(mb + 1) * MB],
                                q_rep[32 * j:32 * (j + 1), ns],
                                start=True, stop=True,
                                tile_position=(32 * j, 0),
                            )
                        if ch == 0:
                            # v^T production rides inside chunk 0, two
                            # m-blocks per 1-bank flex granule (fp8 DoubleRow
                            # over both channel tiles)
                            vpt = flex.tile([P, 2, CH // 2], F32, tag="flex", name=f"vp{g}")
                            for i in range(2):
                                mb = 2 * g + i
                                nc.tensor.matmul(
                                    vpt[:, i, :],
                                    x8[:, mb // 4, :, (mb % 4) * MB:(mb % 4 + 1) * MB],
                                    wvt8w,
                                    start=True, stop=True, perf_mode=DR,
                                )
                            nc.vector.tensor_copy(
                                out=vt8[:, 2 * g:2 * g + 2, :], in_=vpt)
                        if ch == 1 and g < CT:
                            # residual base: fp8 x + bf16 correction +
                            # projection bias (DVE has slack here)
                            f = g
                            nq_ch = NQ // CH
                            nc.vector.scalar_tensor_tensor(
                                out=xqb[f][:, :].rearrange("p (a b) -> p a b", a=nq_ch),
                                in0=x8[:, 0:nq_ch, f, :], scalar=bp_effs[f],
                                in1=xc_sb[f][:, :].rearrange("p (a b) -> p a b", a=nq_ch),
                                op0=mybir.AluOpType.add, op1=mybir.AluOpType.add)
                        pg = pp.tile([P, 2, CH], FP8, tag="pg")
                        nc.scalar.activation(
                            out=pg, in_=stg,
                            func=mybir.ActivationFunctionType.Exp,
                            scale=SM_SCALE,
                        )
                        p_tiles[k] = pg
                        # deferred epilogue of the previous chunk, one output
                        # strip per group so it never swamps one group's slack
                        if g in (4, 5) and pend is not None:
                            emit_epilogue_f(pend, g - 4)
                            if g == 5:
                                pend = None
                    if k >= LAG:
                        kp = k - LAG
                        chp, gp = divmod(kp, NG2)
                        pg = p_tiles[kp]
                        p_tiles[kp] = None
                        # denominator: fp8 DoubleRow ones-matmul, result
                        # replicated across all 128 partitions
                        nc.tensor.matmul(
                            rs_t[chp], ones8, pg,
                            start=(gp == 0), stop=(gp == NG2 - 1),
                            perf_mode=DR,
                        )
                        # P*V: fp8 DoubleRow, two m-blocks per pass
                        for e in range(CT):
                            nc.tensor.matmul(
                                att2_t[chp][:, e, :],
                                vt8[:, 2 * gp:2 * gp + 2, e * P:(e + 1) * P],
                                pg,
                                start=(gp == 0), stop=(gp == NG2 - 1),
                                perf_mode=DR,
                            )
                        if gp == NG2 - 1:
                            nsp = slice(chp * CH, (chp + 1) * CH)
                            pend = eager_epilogue(nsp, att2_t[chp], rs_t[chp])
                for f in range(CT):
                    emit_epilogue_f(pend, f, final=True)

    nc.compile()
    _CACHE["nc"] = nc
    return nc


def qk_query_dma(cn):
    """If x-chunk cn lies in the query half (columns 0:NQ), return its
    local chunk index."""
    return cn if cn < NQ // CH else None


def kernel(x, gamma, beta, wq, bq, wk, bk, wv, bv, wp, bp):
    x = np.ascontiguousarray(np.asarray(x, dtype=np.float32))
    nc = _build()

    GT = G // CT
    ind16 = np.zeros((P, GT), np.float32)
    for c in range(P):
        ind16[c, c // GS] = 1.0 / GS
    indb = np.zeros((GT, P), np.float32)
    for c in range(P):
        indb[c // GS, c] = 1.0

    common = {
        "wqt": np.ascontiguousarray(np.asarray(wq, np.float32).T),
        "wkt": np.ascontiguousarray(np.asarray(wk, np.float32).T),
        "wvt": np.ascontiguousarray(np.asarray(wv, np.float32).T),
        "wpt": np.ascontiguousarray(np.asarray(wp, np.float32).T),
        "gbvp": np.ascontiguousarray(np.stack(
            [np.asarray(gamma, np.float32), np.asarray(beta, np.float32),
             np.asarray(bv, np.float32), np.asarray(bp, np.float32)], axis=1)),
        "bqk": np.ascontiguousarray(np.stack(
            [np.asarray(bq, np.float32), np.asarray(bk, np.float32)], axis=1)),
        "ind16": ind16,
        "indb": indb,
    }

    xf = x.reshape(B, C, N)
    x8all = xf.astype(ml_dtypes.float8_e4m3)
    # bf16 correction x - fp8(x), exact residual reconstruction on device
    xcall = (xf - x8all.astype(np.float32)).astype(ml_dtypes.bfloat16)
    in_maps = []
    for core in range(8):
        b, half = core // 2, core % 2
        m = dict(common)
        # put this core's query tokens in columns 0:NQ (token order within
        # the key axis is irrelevant to GroupNorm stats and softmax sums)
        hs = slice(half * NQ, (half + 1) * NQ)
        if half == 0:
            xp8 = x8all[b]
        else:
            xp8 = np.concatenate([x8all[b][:, NQ:], x8all[b][:, :NQ]], axis=1)
        # device layout [p, chunk, ct, CH] with channel c = ct*128 + p
        m["x8"] = np.ascontiguousarray(
            xp8.reshape(CT, P, N // CH, CH).transpose(1, 2, 0, 3).reshape(P, N * CT))
        m["xc"] = np.ascontiguousarray(xcall[b][:, hs])
        in_maps.append(m)

    global _last_in_maps
    _last_in_maps = in_maps
    res = run_bass_kernel_spmd(nc, in_maps, list(range(8)))

    y = np.empty((B, C, N), np.float32)
    for core in range(8):
        b, half = core // 2, core % 2
        y[b][:, half * NQ:(half + 1) * NQ] = res.results[core]["out"]
    return y.reshape(B, C, H, W)
